# revision 9
# baseline (speedup 1.0000x reference)
"""Trainium2 Bass kernel for nn_NodeBlock (gnn_message_passing).

Computation per graph b (B=512 graphs, N=256 nodes):
  x   = concat([nodes, pooled_edges], -1)              [N, 512]
  xn  = LayerNorm(x) * ln_g + ln_b                     (LN over 512)
  inp = concat([xn, broadcast(globs[b])], -1)          [N, 576]
  scores = MLP_a(inp)  (576->256 SiLU ->4)             [N, 4]
  w   = softmax(mask(scores)/16, axis=N)               [N, 4]
  feats = MLP_f(inp)   (576->1024 SiLU ->256)          [N, 256]
  new_nodes = where(mask, feats, 0) + nodes            [N, 256]
  pooled = sum_n new_nodes * repeat(w, 64)             [256]

Strategy: data-parallel over B across 8 cores (64 graphs/core).
On-chip: LN node-major (bn_stats), PE-transpose to feature-major,
bf16 matmuls (weights stationary), ctx@W1 + biases folded into
per-partition ACT bias, softmax without max-subtraction (exact),
per-head fp32 pooling matmuls. rstd via Ln+Exp so that softmax Exp
and LN rstd share one ACT table set; SiLU set used in main phase
(2 table switches per supergroup of 8 graphs).
"""

import sys

for _p in ("/opt/trn_rl_repo",):
    if _p not in sys.path:
        sys.path.insert(0, _p)

import numpy as np
import ml_dtypes
from contextlib import ExitStack

import concourse.bass as bass
import concourse.bacc as bacc
import concourse.tile as tile
from concourse import mybir
from concourse.bass_utils import run_bass_kernel_spmd

# ----- problem constants (hardcoded per contract) -----
B, N, D = 512, 256, 256
FEAT_IN = 512          # LN dim (nodes + pooled_edges)
GLOB = 64
FHID = 1024
AHID = 256
FOUT = 256
NH = 4
HD = 64
LN_EPS = 1e-5
NCORES = 8
BPC = B // NCORES      # 64 graphs per core
SCALE = 1.0 / np.sqrt(np.float32(FOUT))  # 1/16

AF = mybir.ActivationFunctionType
ALU = mybir.AluOpType
dt = mybir.dt
F32 = dt.float32
BF16 = dt.bfloat16

NT = N // 128          # node tiles per graph (2)
KX = FEAT_IN // 128    # xn K-tiles (4)
MF = FHID // 128       # feat hidden M-tiles (8)
MA = AHID // 128       # attn hidden M-tiles (2)
MO = FOUT // 128       # feat out M-tiles (2)


def build_nc(bpc=BPC, G=8):
    """Build + compile the per-core Bass program (identical on all cores)."""
    assert bpc % G == 0
    NSG = bpc // G

    nc = bacc.Bacc("TRN2", target_bir_lowering=False, debug=False)

    # ---- DRAM I/O ----
    d_nodes = nc.dram_tensor("nodes", [bpc, N, D], F32, kind="ExternalInput").ap()
    d_pedge = nc.dram_tensor("pedges", [bpc, N, D], F32, kind="ExternalInput").ap()
    d_mask = nc.dram_tensor("maskt", [bpc, 128, NT], F32, kind="ExternalInput").ap()
    d_globsT = nc.dram_tensor("globsT", [GLOB, bpc], BF16, kind="ExternalInput").ap()
    d_fw1 = nc.dram_tensor("w_fw1", [FEAT_IN, FHID], BF16, kind="ExternalInput").ap()
    d_fw1c = nc.dram_tensor("w_fw1c", [GLOB, FHID], BF16, kind="ExternalInput").ap()
    d_fw2 = nc.dram_tensor("w_fw2", [FHID, FOUT], BF16, kind="ExternalInput").ap()
    d_aw1 = nc.dram_tensor("w_aw1", [FEAT_IN, AHID], BF16, kind="ExternalInput").ap()
    d_aw1c = nc.dram_tensor("w_aw1c", [GLOB, AHID], BF16, kind="ExternalInput").ap()
    d_aw2 = nc.dram_tensor("w_aw2", [AHID, NH], BF16, kind="ExternalInput").ap()
    d_bf1 = nc.dram_tensor("b_f1", [128, MF], F32, kind="ExternalInput").ap()
    d_bf2 = nc.dram_tensor("b_f2", [128, MO], F32, kind="ExternalInput").ap()
    d_ba1 = nc.dram_tensor("b_a1", [128, MA], F32, kind="ExternalInput").ap()
    d_ident = nc.dram_tensor("ident", [128, 128], BF16, kind="ExternalInput").ap()
    d_ones = nc.dram_tensor("ones_c", [128, 1], F32, kind="ExternalInput").ap()

    d_out_nn = nc.dram_tensor("out_nn", [bpc, N, D], F32, kind="ExternalOutput").ap()
    d_out_pl = nc.dram_tensor("out_pl", [bpc, D], F32, kind="ExternalOutput").ap()

    with tile.TileContext(nc) as tc, ExitStack() as ctx:
        # ---- pools ----
        wp = ctx.enter_context(tc.tile_pool(name="wp", bufs=1))
        xp = ctx.enter_context(tc.tile_pool(name="xp", bufs=2 * G + 4))
        xsp = ctx.enter_context(tc.tile_pool(name="xsp", bufs=3))
        xTp = ctx.enter_context(tc.tile_pool(name="xTp", bufs=3))
        hTp = ctx.enter_context(tc.tile_pool(name="hTp", bufs=2))
        haTp = ctx.enter_context(tc.tile_pool(name="haTp", bufs=2))
        fTp = ctx.enter_context(tc.tile_pool(name="fTp", bufs=2))
        nmp = ctx.enter_context(tc.tile_pool(name="nmp", bufs=3))
        nnp = ctx.enter_context(tc.tile_pool(name="nnp", bufs=G + 2))
        mkp = ctx.enter_context(tc.tile_pool(name="mkp", bufs=G + 4))
        smp = ctx.enter_context(tc.tile_pool(name="smp", bufs=3))   # small stats
        sgp = ctx.enter_context(tc.tile_pool(name="sgp", bufs=2))   # per-supergroup

        pp_x = ctx.enter_context(tc.tile_pool(name="pp_x", bufs=2, space="PSUM"))
        pp_mm = ctx.enter_context(tc.tile_pool(name="pp_mm", bufs=3, space="PSUM"))
        pp_sc = ctx.enter_context(tc.tile_pool(name="pp_sc", bufs=2, space="PSUM"))
        pp_bank = ctx.enter_context(tc.tile_pool(name="pp_bank", bufs=1, space="PSUM"))

        # ---- constants / weights into SBUF ----
        fw1_sb = wp.tile([128, KX, FHID], BF16)
        nc.sync.dma_start(fw1_sb[:], d_fw1.rearrange("(k p) h -> p k h", p=128))
        fw2_sb = wp.tile([128, MF, FOUT], BF16)
        nc.sync.dma_start(fw2_sb[:], d_fw2.rearrange("(k p) h -> p k h", p=128))
        aw1_sb = wp.tile([128, KX, AHID], BF16)
        nc.sync.dma_start(aw1_sb[:], d_aw1.rearrange("(k p) h -> p k h", p=128))
        aw2_sb = wp.tile([128, MA, NH], BF16)
        nc.sync.dma_start(aw2_sb[:], d_aw2.rearrange("(k p) h -> p k h", p=128))
        fw1c_sb = wp.tile([GLOB, FHID], BF16)
        nc.sync.dma_start(fw1c_sb[:], d_fw1c[:, :])
        aw1c_sb = wp.tile([GLOB, AHID], BF16)
        nc.sync.dma_start(aw1c_sb[:], d_aw1c[:, :])
        bf1_sb = wp.tile([128, MF], F32)
        nc.sync.dma_start(bf1_sb[:], d_bf1[:, :])
        bf2_sb = wp.tile([128, MO], F32)
        nc.sync.dma_start(bf2_sb[:], d_bf2[:, :])
        ba1_sb = wp.tile([128, MA], F32)
        nc.sync.dma_start(ba1_sb[:], d_ba1[:, :])
        ident_sb = wp.tile([128, 128], BF16)
        nc.sync.dma_start(ident_sb[:], d_ident[:, :])
        ones_sb = wp.tile([128, 1], F32)
        nc.sync.dma_start(ones_sb[:], d_ones[:, :])
        globsT_sb = wp.tile([GLOB, bpc], BF16)
        nc.sync.dma_start(globsT_sb[:], d_globsT[:, :])

        stage_pl = wp.tile([1, bpc * D], F32)
        eps_sb = wp.tile([128, 1], F32)
        nc.vector.memset(eps_sb[:], LN_EPS)

        # ---- prologue: ctx contributions -> per-graph layer-1 biases ----
        # cvall[:, m*bpc + g] = (globs[g] @ W1c)[m-tile], bias_l1_all = fb1 + cvall
        bias_f1 = wp.tile([128, MF, bpc], F32)
        cv_ps = pp_bank.tile([128, MF * bpc], F32, name="bank_cv", tag="bank_cv")
        for m in range(MF):
            nc.tensor.matmul(
                cv_ps[:, m * bpc:(m + 1) * bpc],
                fw1c_sb[:, m * 128:(m + 1) * 128],
                globsT_sb[:, :],
                start=True, stop=True,
            )
        for m in range(MF):
            nc.vector.tensor_scalar(
                bias_f1[:, m, :], cv_ps[:, m * bpc:(m + 1) * bpc],
                bf1_sb[:, m:m + 1], None, ALU.add,
            )
        bias_a1 = wp.tile([128, MA, bpc], F32)
        cva_ps = pp_bank.tile([128, MA * bpc], F32, name="bank_cv", tag="bank_cv")
        for m in range(MA):
            nc.tensor.matmul(
                cva_ps[:, m * bpc:(m + 1) * bpc],
                aw1c_sb[:, m * 128:(m + 1) * 128],
                globsT_sb[:, :],
                start=True, stop=True,
            )
        for m in range(MA):
            nc.vector.tensor_scalar(
                bias_a1[:, m, :], cva_ps[:, m * bpc:(m + 1) * bpc],
                ba1_sb[:, m:m + 1], None, ALU.add,
            )

        # ---- per-supergroup state ----
        x_tiles = {}      # (gabs, t) -> x tile [128, 512] f32
        mask_tiles = {}   # gabs -> [128, NT] f32
        nn_tiles = {}     # gabs -> [128, NT, D] f32
        aggr_tiles = {}   # sg -> [128, G, NT, 2] f32 (mean, var)
        rstd_tiles = {}   # sg -> [128, G*NT] f32
        sc_ps_tiles = {}  # sg -> psum scores [128, G*2*NH]
        e_tiles = {}      # sg -> e_all [128, G*2*NH] f32

        def load_and_stats(sg):
            """DMA x for supergroup sg and compute LN stats."""
            aggr = sgp.tile([128, G, NT, 2], F32, name=f"aggr{sg % 2}", tag="aggr")
            aggr_tiles[sg] = aggr
            for j in range(G):
                gabs = sg * G + j
                for t in range(NT):
                    xt = xp.tile([128, FEAT_IN], F32, name=f"x{gabs}_{t}", tag="x")
                    x_tiles[(gabs, t)] = xt
                    nc.sync.dma_start(xt[:, 0:D], d_nodes[gabs, t * 128:(t + 1) * 128, :])
                    nc.sync.dma_start(xt[:, D:2 * D], d_pedge[gabs, t * 128:(t + 1) * 128, :])
                    bn6 = smp.tile([128, 6], F32, name="bn6", tag="bn6")
                    nc.vector.bn_stats(bn6[:], xt[:])
                    nc.vector.bn_aggr(aggr[:, j, t, :], bn6[:])
                mt = mkp.tile([128, NT], F32, name=f"mk{gabs}", tag="mk")
                mask_tiles[gabs] = mt
                nc.sync.dma_start(mt[:], d_mask[gabs, :, :])

        def phase_rstd(sg):
            """rstd = exp(-0.5 * ln(var + eps)) for all graphs of sg."""
            aggr = aggr_tiles[sg]
            lnv = sgp.tile([128, G, NT, 1], F32, name=f"lnv{sg % 2}", tag="lnv")
            nc.scalar.activation(lnv[:], aggr[:, :, :, 1:2], AF.Ln, bias=eps_sb[:, 0:1])
            rstd = sgp.tile([128, G, NT, 1], F32, name=f"rstd{sg % 2}", tag="rstd")
            nc.scalar.activation(rstd[:], lnv[:], AF.Exp, scale=-0.5)
            rstd_tiles[sg] = rstd

        def phase_softmax(sg):
            """Exp of staged scores for supergroup sg (one ACT op)."""
            e_all = sgp.tile([128, G * NT * NH], F32, name=f"eall{sg % 2}", tag="eall")
            nc.scalar.activation(e_all[:], sc_ps_tiles[sg][:], AF.Exp, scale=float(SCALE))
            e_tiles[sg] = e_all
            del sc_ps_tiles[sg]

        def pool_graph(sg, j):
            """Masked softmax normalization + attention pooling for one graph."""
            gabs = sg * G + j
            e_all = e_tiles[sg]
            mt = mask_tiles.pop(gabs)
            nn = nn_tiles.pop(gabs)
            em = smp.tile([128, NT * NH], F32, name="em", tag="em")
            for t in range(NT):
                nc.vector.tensor_scalar(
                    em[:, t * NH:(t + 1) * NH],
                    e_all[:, (j * NT + t) * NH:(j * NT + t + 1) * NH],
                    mt[:, t:t + 1], None, ALU.mult,
                )
            bank = pp_bank.tile([128, 512], F32, name="bank", tag="bank_cv")
            s_ap = bank[0:1, 0:NH]
            pr_ap = bank[0:1, 256:256 + D]
            for t in range(NT):
                nc.tensor.matmul(
                    s_ap, ones_sb[:, 0:1], em[:, t * NH:(t + 1) * NH],
                    start=(t == 0), stop=(t == NT - 1),
                )
            rs = smp.tile([1, NH], F32, name="rs", tag="rs")
            nc.vector.reciprocal(rs[:], s_ap)
            for h in range(NH):
                for t in range(NT):
                    last = (h == NH - 1) and (t == NT - 1)
                    nc.tensor.matmul(
                        pr_ap[:, h * HD:(h + 1) * HD],
                        em[:, t * NH + h:t * NH + h + 1],
                        nn[:, t, h * HD:(h + 1) * HD],
                        start=False, stop=False, skip_group_check=True,
                    )
            for h in range(NH):
                nc.vector.tensor_scalar(
                    stage_pl[0:1, gabs * D + h * HD:gabs * D + (h + 1) * HD],
                    pr_ap[:, h * HD:(h + 1) * HD],
                    rs[0:1, h:h + 1], None, ALU.mult,
                )

        def main_graph(sg, j, sc_ps):
            """LN-normalize, transpose, MLPs, residual, scores for one graph."""
            gabs = sg * G + j
            aggr = aggr_tiles[sg]
            rstd = rstd_tiles[sg]

            # normalize + transpose -> xT [128, KX, N] bf16 (feature-major)
            xT = xTp.tile([128, KX, N], BF16, name="xT", tag="xT")
            for t in range(NT):
                xt = x_tiles[(gabs, t)]
                xs = xsp.tile([128, FEAT_IN], BF16, name="xs", tag="xs")
                nc.vector.tensor_scalar(
                    xs[:], xt[:],
                    aggr[:, j, t, 0:1], rstd[:, j, t, 0:1],
                    ALU.subtract, ALU.mult,
                )
                ps_x = pp_x.tile([128, FEAT_IN], BF16, name="ps_x", tag="ps_x")
                for k in range(KX):
                    nc.tensor.transpose(
                        ps_x[:, k * 128:(k + 1) * 128],
                        xs[:, k * 128:(k + 1) * 128],
                        ident_sb[:],
                    )
                nc.scalar.activation(
                    xT[:, :, t * 128:(t + 1) * 128],
                    ps_x[:].rearrange("p (k n) -> p k n", k=KX), AF.Copy,
                )

            # feat MLP layer 1 + SiLU: hT [128, MF, N] bf16
            hT = hTp.tile([128, MF, N], BF16, name="hT", tag="hT")
            for m in range(MF):
                z1 = pp_mm.tile([128, N], F32, name="z1", tag="zz")
                for k in range(KX):
                    nc.tensor.matmul(
                        z1[:], fw1_sb[:, k, m * 128:(m + 1) * 128], xT[:, k, :],
                        start=(k == 0), stop=(k == KX - 1),
                    )
                nc.scalar.activation(
                    hT[:, m, :], z1[:], AF.Silu, bias=bias_f1[:, m, gabs:gabs + 1],
                )

            # attn MLP layer 1 + SiLU: haT [128, MA, N] bf16
            haT = haTp.tile([128, MA, N], BF16, name="haT", tag="haT")
            for m in range(MA):
                za = pp_mm.tile([128, N], F32, name="za", tag="zz")
                for k in range(KX):
                    nc.tensor.matmul(
                        za[:], aw1_sb[:, k, m * 128:(m + 1) * 128], xT[:, k, :],
                        start=(k == 0), stop=(k == KX - 1),
                    )
                nc.scalar.activation(
                    haT[:, m, :], za[:], AF.Silu, bias=bias_a1[:, m, gabs:gabs + 1],
                )

            # feat MLP layer 2 (+bias): fT [128, MO, N] bf16
            fT = fTp.tile([128, MO, N], BF16, name="fT", tag="fT")
            for m in range(MO):
                z2 = pp_mm.tile([128, N], F32, name="z2", tag="zz")
                for k in range(MF):
                    nc.tensor.matmul(
                        z2[:], fw2_sb[:, k, m * 128:(m + 1) * 128], hT[:, k, :],
                        start=(k == 0), stop=(k == MF - 1),
                    )
                nc.scalar.activation(
                    fT[:, m, :], z2[:], AF.Identity, bias=bf2_sb[:, m:m + 1],
                )

            # attn layer 2, node-major scores -> shared psum (per supergroup)
            for t in range(NT):
                for k in range(MA):
                    nc.tensor.matmul(
                        sc_ps[:, (j * NT + t) * NH:(j * NT + t + 1) * NH],
                        haT[:, k, t * 128:(t + 1) * 128],
                        aw2_sb[:, k, :],
                        start=(k == 0), stop=(k == MA - 1),
                    )

            # transpose feats back to node-major, mask + residual -> nn
            mt = mask_tiles[gabs]
            nn = nnp.tile([128, NT, D], F32, name="nn", tag="nn")
            nn_tiles[gabs] = nn
            for t in range(NT):
                ps_nn = pp_mm.tile([128, D], BF16, name="ps_nn", tag="zz")
                for f in range(MO):
                    nc.tensor.transpose(
                        ps_nn[:, f * 128:(f + 1) * 128],
                        fT[:, f, t * 128:(t + 1) * 128],
                        ident_sb[:],
                    )
                nmk = nmp.tile([128, D], F32, name="nmk", tag="nmk")
                nc.scalar.activation(nmk[:], ps_nn[:], AF.Copy, scale=mt[:, t:t + 1])
                xt = x_tiles.pop((gabs, t))
                nc.vector.tensor_tensor(nn[:, t, :], nmk[:], xt[:, 0:D], ALU.add)
            nc.sync.dma_start(
                d_out_nn[gabs].rearrange("(t p) d -> p t d", p=128), nn[:]
            )

        # ---- pipeline ----
        load_and_stats(0)
        for sg in range(NSG + 1):
            # transcendental phase (natural_log_exp table set)
            if sg > 0:
                phase_softmax(sg - 1)
            if sg < NSG:
                phase_rstd(sg)
            if sg > 0:
                for j in range(G):
                    pool_graph(sg - 1, j)
            # main phase (silu table set)
            if sg < NSG:
                sc_ps = pp_sc.tile([128, G * NT * NH], F32, name=f"scps{sg % 2}", tag="scps")
                sc_ps_tiles[sg] = sc_ps
                for j in range(G):
                    main_graph(sg, j, sc_ps)
                if sg + 1 < NSG:
                    load_and_stats(sg + 1)

        nc.sync.dma_start(d_out_pl.rearrange("b d -> () (b d)"), stage_pl[:])

    nc.compile()
    return nc


def prep_inputs(nodes, pooled_edges, mask, globs, ln_g, ln_b,
                fw1, fb1, fw2, fb2, aw1, ab1, aw2, ab2, bpc=BPC, ncores=NCORES):
    """Host-side prep: fold LN affine into W1s, reshape biases/masks, cast."""
    f32 = np.float32
    bf = ml_dtypes.bfloat16
    ln_g = np.asarray(ln_g, f32)
    ln_b = np.asarray(ln_b, f32)
    fw1 = np.asarray(fw1, f32)
    aw1 = np.asarray(aw1, f32)

    fw1_eff = (fw1[:FEAT_IN] * ln_g[:, None]).astype(bf)
    fw1c = np.ascontiguousarray(fw1[FEAT_IN:]).astype(bf)
    fb1_eff = (np.asarray(fb1, f32) + ln_b @ fw1[:FEAT_IN]).astype(f32)
    aw1_eff = (aw1[:FEAT_IN] * ln_g[:, None]).astype(bf)
    aw1c = np.ascontiguousarray(aw1[FEAT_IN:]).astype(bf)
    ab1_eff = (np.asarray(ab1, f32) + ln_b @ aw1[:FEAT_IN]).astype(f32)
    # ab2 cancels exactly in the softmax; dropped.

    shared = {
        "w_fw1": fw1_eff,
        "w_fw1c": fw1c,
        "w_fw2": np.asarray(fw2, f32).astype(bf),
        "w_aw1": aw1_eff,
        "w_aw1c": aw1c,
        "w_aw2": np.asarray(aw2, f32).astype(bf),
        "b_f1": np.ascontiguousarray(fb1_eff.reshape(MF, 128).T),
        "b_f2": np.ascontiguousarray(np.asarray(fb2, f32).reshape(MO, 128).T),
        "b_a1": np.ascontiguousarray(ab1_eff.reshape(MA, 128).T),
        "ident": np.eye(128, dtype=bf),
        "ones_c": np.ones((128, 1), f32),
    }

    mask_t = np.ascontiguousarray(
        np.asarray(mask).astype(f32).reshape(-1, NT, 128).transpose(0, 2, 1)
    )
    globs = np.asarray(globs, f32)
    nodes = np.asarray(nodes, f32)
    pooled_edges = np.asarray(pooled_edges, f32)

    in_maps = []
    for c in range(ncores):
        s = slice(c * bpc, (c + 1) * bpc)
        m = dict(shared)
        m["nodes"] = np.ascontiguousarray(nodes[s])
        m["pedges"] = np.ascontiguousarray(pooled_edges[s])
        m["maskt"] = np.ascontiguousarray(mask_t[s])
        m["globsT"] = np.ascontiguousarray(globs[s].T).astype(bf)
        in_maps.append(m)
    return in_maps


_NC_CACHE = {}


def _get_nc(bpc=BPC, G=8):
    key = (bpc, G)
    if key not in _NC_CACHE:
        _NC_CACHE[key] = build_nc(bpc, G)
    return _NC_CACHE[key]


def kernel(nodes, pooled_edges, mask, globs, ln_g, ln_b,
           fw1, fb1, fw2, fb2, aw1, ab1, aw2, ab2):
    nc = _get_nc()
    in_maps = prep_inputs(nodes, pooled_edges, mask, globs, ln_g, ln_b,
                          fw1, fb1, fw2, fb2, aw1, ab1, aw2, ab2)
    res = run_bass_kernel_spmd(nc, in_maps, list(range(NCORES)))
    new_nodes = np.concatenate([r["out_nn"] for r in res.results], axis=0)
    pooled = np.concatenate([r["out_pl"] for r in res.results], axis=0)
    return new_nodes, pooled


# revision 19
# speedup vs baseline: 79532.6344x; 79532.6344x over previous
"""Trainium2 Bass kernel for nn_NodeBlock (gnn_message_passing).

Computation per graph b (B=512 graphs, N=256 nodes):
  x   = concat([nodes, pooled_edges], -1)              [N, 512]
  xn  = LayerNorm(x) * ln_g + ln_b                     (LN over 512)
  inp = concat([xn, broadcast(globs[b])], -1)          [N, 576]
  scores = MLP_a(inp)  (576->256 SiLU ->4)             [N, 4]
  w   = softmax(mask(scores)/16, axis=N)               [N, 4]
  feats = MLP_f(inp)   (576->1024 SiLU ->256)          [N, 256]
  new_nodes = where(mask, feats, 0) + nodes            [N, 256]
  pooled = sum_n new_nodes * repeat(w, 64)             [256]

Strategy: data-parallel over B across 8 cores (64 graphs/core).
On-chip: LN node-major (bn_stats), PE-transpose to feature-major,
bf16 matmuls (weights stationary), ctx@W1 + biases folded into
per-partition ACT bias, softmax without max-subtraction (exact),
per-head fp32 pooling matmuls. rstd via Ln+Exp so that softmax Exp
and LN rstd share one ACT table set; SiLU set used in main phase
(2 table switches per supergroup of 8 graphs).
"""

import sys

for _p in ("/opt/trn_rl_repo",):
    if _p not in sys.path:
        sys.path.insert(0, _p)

import numpy as np
import ml_dtypes
from contextlib import ExitStack

import concourse.bass as bass
import concourse.bacc as bacc
import concourse.tile as tile
from concourse.tile import add_dep_helper
from concourse import mybir
from concourse.bass_utils import run_bass_kernel_spmd

# Steer the ACT table-load inserter to the combined natural_log_exp set for
# Ln and Exp (otherwise it alternates natural_log / exp_and_others, paying an
# extra ~2.7us table load per transition). Membership is edited in the copy
# handed to the chooser only; set IDs still index the real act_info.json.
_orig_get_act_tables = bacc.get_activation_tables


def _patched_get_act_tables(arch):
    tabs = {k: set(v) for k, v in _orig_get_act_tables(arch).items()}
    _ln = mybir.ActivationFunctionType.Ln
    _exp = mybir.ActivationFunctionType.Exp
    for name, funcs in tabs.items():
        if name != "natural_log_exp_and_others":
            funcs.discard(_ln)
            funcs.discard(_exp)
    return tabs


bacc.get_activation_tables = _patched_get_act_tables

# ----- problem constants (hardcoded per contract) -----
B, N, D = 512, 256, 256
FEAT_IN = 512          # LN dim (nodes + pooled_edges)
GLOB = 64
FHID = 1024
AHID = 256
FOUT = 256
NH = 4
HD = 64
LN_EPS = 1e-5
NCORES = 8
BPC = B // NCORES      # 64 graphs per core
SCALE = 1.0 / np.sqrt(np.float32(FOUT))  # 1/16

AF = mybir.ActivationFunctionType
ALU = mybir.AluOpType
dt = mybir.dt
F32 = dt.float32
BF16 = dt.bfloat16

NT = N // 128          # node tiles per graph (2)
KX = FEAT_IN // 128    # xn K-tiles (4)
MF = FHID // 128       # feat hidden M-tiles (8)
MA = AHID // 128       # attn hidden M-tiles (2)
MO = FOUT // 128       # feat out M-tiles (2)


def build_nc(bpc=BPC, G=8):
    """Build + compile the per-core Bass program (identical on all cores)."""
    assert bpc % G == 0
    NSG = bpc // G

    nc = bacc.Bacc("TRN2", target_bir_lowering=False, debug=False)

    # ---- DRAM I/O ----
    d_nodes = nc.dram_tensor("nodes", [bpc, N, D], F32, kind="ExternalInput").ap()
    d_pedge = nc.dram_tensor("pedges", [bpc, N, D], F32, kind="ExternalInput").ap()
    d_mask = nc.dram_tensor("maskt", [bpc, 128, NT], F32, kind="ExternalInput").ap()
    d_globsT = nc.dram_tensor("globsT", [GLOB, bpc], BF16, kind="ExternalInput").ap()
    d_fw1 = nc.dram_tensor("w_fw1", [FEAT_IN, FHID], BF16, kind="ExternalInput").ap()
    d_fw1c = nc.dram_tensor("w_fw1c", [GLOB, FHID], BF16, kind="ExternalInput").ap()
    d_fw2 = nc.dram_tensor("w_fw2", [FHID, FOUT], BF16, kind="ExternalInput").ap()
    d_aw1 = nc.dram_tensor("w_aw1", [FEAT_IN, AHID], BF16, kind="ExternalInput").ap()
    d_aw1c = nc.dram_tensor("w_aw1c", [GLOB, AHID], BF16, kind="ExternalInput").ap()
    d_aw2 = nc.dram_tensor("w_aw2", [AHID, NH], BF16, kind="ExternalInput").ap()
    d_bf1 = nc.dram_tensor("b_f1", [128, MF], F32, kind="ExternalInput").ap()
    d_bf2 = nc.dram_tensor("b_f2", [128, MO], F32, kind="ExternalInput").ap()
    d_ba1 = nc.dram_tensor("b_a1", [128, MA], F32, kind="ExternalInput").ap()
    d_ident = nc.dram_tensor("ident", [128, 128], BF16, kind="ExternalInput").ap()
    d_ones = nc.dram_tensor("ones_c", [128, 1], F32, kind="ExternalInput").ap()

    d_out_nn = nc.dram_tensor("out_nn", [bpc, N, D], F32, kind="ExternalOutput").ap()
    d_out_pl = nc.dram_tensor("out_pl", [bpc, D], F32, kind="ExternalOutput").ap()

    with tile.TileContext(nc) as tc, ExitStack() as ctx:
        # ---- pools ----
        wp = ctx.enter_context(tc.tile_pool(name="wp", bufs=1))
        xp = ctx.enter_context(tc.tile_pool(name="xp", bufs=G + 2))
        xsp = ctx.enter_context(tc.tile_pool(name="xsp", bufs=3))
        xTp = ctx.enter_context(tc.tile_pool(name="xTp", bufs=3))
        hTp = ctx.enter_context(tc.tile_pool(name="hTp", bufs=2))
        haTp = ctx.enter_context(tc.tile_pool(name="haTp", bufs=2))
        fTp = ctx.enter_context(tc.tile_pool(name="fTp", bufs=2))
        nmp = ctx.enter_context(tc.tile_pool(name="nmp", bufs=3))
        nnp = ctx.enter_context(tc.tile_pool(name="nnp", bufs=G + 2))
        mkp = ctx.enter_context(tc.tile_pool(name="mkp", bufs=2))
        smp = ctx.enter_context(tc.tile_pool(name="smp", bufs=3))   # small stats
        sgp = ctx.enter_context(tc.tile_pool(name="sgp", bufs=2))   # per-supergroup

        pp_x = ctx.enter_context(tc.tile_pool(name="pp_x", bufs=2, space="PSUM"))
        pp_mm = ctx.enter_context(tc.tile_pool(name="pp_mm", bufs=3, space="PSUM"))
        pp_sc = ctx.enter_context(tc.tile_pool(name="pp_sc", bufs=2, space="PSUM"))
        pp_bank = ctx.enter_context(tc.tile_pool(name="pp_bank", bufs=1, space="PSUM"))

        # ---- constants / weights into SBUF ----
        fw1_sb = wp.tile([128, KX, FHID], BF16)
        nc.sync.dma_start(fw1_sb[:], d_fw1.rearrange("(k p) h -> p k h", p=128))
        fw2_sb = wp.tile([128, MF, FOUT], BF16)
        nc.sync.dma_start(fw2_sb[:], d_fw2.rearrange("(k p) h -> p k h", p=128))
        aw1_sb = wp.tile([128, KX, AHID], BF16)
        nc.sync.dma_start(aw1_sb[:], d_aw1.rearrange("(k p) h -> p k h", p=128))
        aw2_sb = wp.tile([128, MA, NH], BF16)
        nc.sync.dma_start(aw2_sb[:], d_aw2.rearrange("(k p) h -> p k h", p=128))
        fw1c_sb = wp.tile([GLOB, FHID], BF16)
        nc.sync.dma_start(fw1c_sb[:], d_fw1c[:, :])
        aw1c_sb = wp.tile([GLOB, AHID], BF16)
        nc.sync.dma_start(aw1c_sb[:], d_aw1c[:, :])
        bf1_sb = wp.tile([128, MF], F32)
        nc.sync.dma_start(bf1_sb[:], d_bf1[:, :])
        bf2_sb = wp.tile([128, MO], F32)
        nc.sync.dma_start(bf2_sb[:], d_bf2[:, :])
        ba1_sb = wp.tile([128, MA], F32)
        nc.sync.dma_start(ba1_sb[:], d_ba1[:, :])
        ident_sb = wp.tile([128, 128], BF16)
        nc.sync.dma_start(ident_sb[:], d_ident[:, :])
        ones_sb = wp.tile([128, 1], F32)
        nc.sync.dma_start(ones_sb[:], d_ones[:, :])
        globsT_sb = wp.tile([GLOB, bpc], BF16)
        nc.sync.dma_start(globsT_sb[:], d_globsT[:, :])

        stage_pl = wp.tile([1, bpc * D], F32)
        eps_sb = wp.tile([128, 1], F32)
        nc.vector.memset(eps_sb[:], LN_EPS)

        # ---- prologue: ctx contributions -> per-graph layer-1 biases ----
        # cvall[:, m*bpc + g] = (globs[g] @ W1c)[m-tile], bias_l1_all = fb1 + cvall
        bias_f1 = wp.tile([128, MF, bpc], F32)
        cv_ps = pp_bank.tile([128, MF * bpc], F32, name="bank_cv", tag="bank_cv")
        for m in range(MF):
            nc.tensor.matmul(
                cv_ps[:, m * bpc:(m + 1) * bpc],
                fw1c_sb[:, m * 128:(m + 1) * 128],
                globsT_sb[:, :],
                start=True, stop=True,
            )
        for m in range(MF):
            nc.vector.tensor_scalar(
                bias_f1[:, m, :], cv_ps[:, m * bpc:(m + 1) * bpc],
                bf1_sb[:, m:m + 1], None, ALU.add,
            )
        bias_a1 = wp.tile([128, MA, bpc], F32)
        cva_ps = pp_bank.tile([128, MA * bpc], F32, name="bank_cv", tag="bank_cv")
        for m in range(MA):
            nc.tensor.matmul(
                cva_ps[:, m * bpc:(m + 1) * bpc],
                aw1c_sb[:, m * 128:(m + 1) * 128],
                globsT_sb[:, :],
                start=True, stop=True,
            )
        for m in range(MA):
            nc.vector.tensor_scalar(
                bias_a1[:, m, :], cva_ps[:, m * bpc:(m + 1) * bpc],
                ba1_sb[:, m:m + 1], None, ALU.add,
            )

        # ---- per-supergroup state ----
        x_tiles = {}      # (gabs, t) -> x tile [128, 512] f32
        mask_tiles = {}   # gabs -> [128, NT] f32
        nn_tiles = {}     # gabs -> [128, NT, D] f32
        aggr_tiles = {}   # sg -> [128, G, NT, 2] f32 (mean, var)
        rstd_tiles = {}   # sg -> [128, G*NT] f32
        sc_ps_tiles = {}  # sg -> psum scores [128, G*2*NH]
        e_tiles = {}      # sg -> e_all [128, G*2*NH] f32
        act_insts = {}    # sg -> list of main-phase ACT instructions

        def gate_after_main(inst, sg_prev):
            # Keep phase-S transcendentals from interleaving with the previous
            # main phase's Silu ops (each interleave costs a ~2.7us ACT
            # table-set load).
            for prior in act_insts.get(sg_prev, ()):
                add_dep_helper(inst.ins, prior.ins, sync=False,
                               reason="ACT table-set phase ordering")

        def load_and_stats(sg):
            """DMA x for supergroup sg and compute LN stats."""
            aggr = sgp.tile([128, G, NT, 2], F32, name=f"aggr{sg % 2}", tag="aggr")
            aggr_tiles[sg] = aggr
            msk = mkp.tile([128, G, NT], F32, name=f"mk{sg % 2}", tag="mk")
            mask_tiles[sg] = msk
            nc.sync.dma_start(
                msk[:], d_mask[sg * G:(sg + 1) * G].rearrange("g p t -> p g t"))
            for j in range(G):
                gabs = sg * G + j
                xt = xp.tile([128, NT, FEAT_IN], F32, name=f"x{gabs}", tag="x")
                x_tiles[gabs] = xt
                nc.sync.dma_start(
                    xt[:, :, 0:D], d_nodes[gabs].rearrange("(t p) d -> p t d", p=128))
                nc.sync.dma_start(
                    xt[:, :, D:2 * D], d_pedge[gabs].rearrange("(t p) d -> p t d", p=128))
                for t in range(NT):
                    bn6 = smp.tile([128, 6], F32, name="bn6", tag="bn6")
                    nc.vector.bn_stats(bn6[:], xt[:, t, :])
                    nc.vector.bn_aggr(aggr[:, j, t, :], bn6[:])

        def phase_rstd(sg):
            """rstd = exp(-0.5 * ln(var + eps)) for all graphs of sg."""
            aggr = aggr_tiles[sg]
            lnv = sgp.tile([128, G, NT, 1], F32, name=f"lnv{sg % 2}", tag="lnv")
            i1 = nc.scalar.activation(lnv[:], aggr[:, :, :, 1:2], AF.Ln, bias=eps_sb[:, 0:1])
            rstd = sgp.tile([128, G, NT, 1], F32, name=f"rstd{sg % 2}", tag="rstd")
            i2 = nc.scalar.activation(rstd[:], lnv[:], AF.Exp, scale=-0.5)
            gate_after_main(i1, sg - 1)
            gate_after_main(i2, sg - 1)
            rstd_tiles[sg] = rstd

        def phase_softmax(sg):
            """Exp of staged scores for supergroup sg (one ACT op)."""
            e_all = sgp.tile([128, G * NT * NH], F32, name=f"eall{sg % 2}", tag="eall")
            i1 = nc.scalar.activation(e_all[:], sc_ps_tiles[sg][:], AF.Exp, scale=float(SCALE))
            gate_after_main(i1, sg)
            e_tiles[sg] = e_all
            del sc_ps_tiles[sg]

        def pool_graph(sg, j):
            """Masked softmax normalization + attention pooling for one graph."""
            gabs = sg * G + j
            e_all = e_tiles[sg]
            msk = mask_tiles[sg]
            nn = nn_tiles.pop(gabs)
            em = smp.tile([128, NT * NH], F32, name="em", tag="em")
            for t in range(NT):
                nc.vector.tensor_scalar(
                    em[:, t * NH:(t + 1) * NH],
                    e_all[:, (j * NT + t) * NH:(j * NT + t + 1) * NH],
                    msk[:, j, t:t + 1], None, ALU.mult,
                )
            bank = pp_bank.tile([128, 512], F32, name="bank", tag="bank_cv")
            s_ap = bank[0:1, 0:NH]
            pr_ap = bank[0:1, 256:256 + D]
            for t in range(NT):
                nc.tensor.matmul(
                    s_ap, ones_sb[:, 0:1], em[:, t * NH:(t + 1) * NH],
                    start=(t == 0), stop=(t == NT - 1),
                )
            rs = smp.tile([1, NH], F32, name="rs", tag="rs")
            nc.vector.reciprocal(rs[:], s_ap)
            for h in range(NH):
                for t in range(NT):
                    last = (h == NH - 1) and (t == NT - 1)
                    nc.tensor.matmul(
                        pr_ap[:, h * HD:(h + 1) * HD],
                        em[:, t * NH + h:t * NH + h + 1],
                        nn[:, t, h * HD:(h + 1) * HD],
                        start=False, stop=False, skip_group_check=True,
                    )
            for h in range(NH):
                nc.vector.tensor_scalar(
                    stage_pl[0:1, gabs * D + h * HD:gabs * D + (h + 1) * HD],
                    pr_ap[:, h * HD:(h + 1) * HD],
                    rs[0:1, h:h + 1], None, ALU.mult,
                )

        NP = 2 * N  # paired free dim (two graphs' nodes)

        def main_pair(sg, jp, sc_ps):
            """LN, transpose, MLPs, residual, scores for a PAIR of graphs.

            Pairing doubles the matmul moving dim to 512, halving the
            LDWEIGHTS count per graph (the PE-time bottleneck)."""
            js = (2 * jp, 2 * jp + 1)
            gabss = tuple(sg * G + j for j in js)
            aggr = aggr_tiles[sg]
            rstd = rstd_tiles[sg]
            acts = act_insts.setdefault(sg, [])

            # normalize + transpose -> xT [128, KX, NP] bf16 (feature-major)
            # pair columns: [g0t0, g0t1, g1t0, g1t1] x 128
            xT = xTp.tile([128, KX, NP], BF16, name="xT", tag="xT")
            for gi, (j, gabs) in enumerate(zip(js, gabss)):
                xt = x_tiles[gabs]
                for t in range(NT):
                    xs = xsp.tile([128, FEAT_IN], BF16, name="xs", tag="xs")
                    nc.vector.tensor_scalar(
                        xs[:], xt[:, t, :],
                        aggr[:, j, t, 0:1], rstd[:, j, t, 0:1],
                        ALU.subtract, ALU.mult,
                    )
                    ps_x = pp_x.tile([128, FEAT_IN], BF16, name="ps_x", tag="ps_x")
                    for k in range(KX):
                        nc.tensor.transpose(
                            ps_x[:, k * 128:(k + 1) * 128],
                            xs[:, k * 128:(k + 1) * 128],
                            ident_sb[:],
                        )
                    off = (gi * NT + t) * 128
                    nc.scalar.activation(
                        xT[:, :, off:off + 128],
                        ps_x[:].rearrange("p (k n) -> p k n", k=KX), AF.Copy,
                    )

            # feat MLP layer 1 + SiLU: hT [128, MF, NP] bf16
            hT = hTp.tile([128, MF, NP], BF16, name="hT", tag="hT")
            for m in range(MF):
                z1 = pp_mm.tile([128, NP], F32, name="z1", tag="zz")
                for k in range(KX):
                    nc.tensor.matmul(
                        z1[:], fw1_sb[:, k, m * 128:(m + 1) * 128], xT[:, k, :],
                        start=(k == 0), stop=(k == KX - 1),
                    )
                for gi, gabs in enumerate(gabss):
                    acts.append(nc.scalar.activation(
                        hT[:, m, gi * N:(gi + 1) * N], z1[:, gi * N:(gi + 1) * N],
                        AF.Silu, bias=bias_f1[:, m, gabs:gabs + 1],
                    ))

            # attn MLP layer 1 + SiLU: haT [128, MA, NP] bf16
            haT = haTp.tile([128, MA, NP], BF16, name="haT", tag="haT")
            for m in range(MA):
                za = pp_mm.tile([128, NP], F32, name="za", tag="zz")
                for k in range(KX):
                    nc.tensor.matmul(
                        za[:], aw1_sb[:, k, m * 128:(m + 1) * 128], xT[:, k, :],
                        start=(k == 0), stop=(k == KX - 1),
                    )
                for gi, gabs in enumerate(gabss):
                    acts.append(nc.scalar.activation(
                        haT[:, m, gi * N:(gi + 1) * N], za[:, gi * N:(gi + 1) * N],
                        AF.Silu, bias=bias_a1[:, m, gabs:gabs + 1],
                    ))

            # feat MLP layer 2 (+bias, graph-independent): fT [128, MO, NP] bf16
            fT = fTp.tile([128, MO, NP], BF16, name="fT", tag="fT")
            for m in range(MO):
                z2 = pp_mm.tile([128, NP], F32, name="z2", tag="zz")
                for k in range(MF):
                    nc.tensor.matmul(
                        z2[:], fw2_sb[:, k, m * 128:(m + 1) * 128], hT[:, k, :],
                        start=(k == 0), stop=(k == MF - 1),
                    )
                acts.append(nc.scalar.activation(
                    fT[:, m, :], z2[:], AF.Identity, bias=bf2_sb[:, m:m + 1],
                ))

            # attn layer 2, node-major scores -> shared psum (per supergroup)
            for gi, j in enumerate(js):
                for t in range(NT):
                    for k in range(MA):
                        nc.tensor.matmul(
                            sc_ps[:, (j * NT + t) * NH:(j * NT + t + 1) * NH],
                            haT[:, k, (gi * NT + t) * 128:(gi * NT + t + 1) * 128],
                            aw2_sb[:, k, :],
                            start=(k == 0), stop=(k == MA - 1),
                        )

            # transpose feats back to node-major, mask + residual -> nn
            for gi, (j, gabs) in enumerate(zip(js, gabss)):
                msk = mask_tiles[sg]
                nn = nnp.tile([128, NT, D], F32, name="nn", tag="nn")
                nn_tiles[gabs] = nn
                xt = x_tiles.pop(gabs)
                for t in range(NT):
                    ps_nn = pp_mm.tile([128, D], BF16, name="ps_nn", tag="zz")
                    for f in range(MO):
                        nc.tensor.transpose(
                            ps_nn[:, f * 128:(f + 1) * 128],
                            fT[:, f, (gi * NT + t) * 128:(gi * NT + t + 1) * 128],
                            ident_sb[:],
                        )
                    nmk = nmp.tile([128, D], F32, name="nmk", tag="nmk")
                    nc.vector.tensor_scalar(
                        nmk[:], ps_nn[:], msk[:, j, t:t + 1], None, ALU.mult,
                    )
                    nc.vector.tensor_tensor(nn[:, t, :], nmk[:], xt[:, t, 0:D], ALU.add)
                nc.sync.dma_start(
                    d_out_nn[gabs].rearrange("(t p) d -> p t d", p=128), nn[:]
                )

        # ---- pipeline ----
        load_and_stats(0)
        for sg in range(NSG + 1):
            # transcendental phase (natural_log_exp table set)
            if sg > 0:
                phase_softmax(sg - 1)
            if sg < NSG:
                phase_rstd(sg)
            if sg > 0:
                for j in range(G):
                    pool_graph(sg - 1, j)
            # main phase (silu table set)
            if sg < NSG:
                sc_ps = pp_sc.tile([128, G * NT * NH], F32, name=f"scps{sg % 2}", tag="scps")
                sc_ps_tiles[sg] = sc_ps
                for jp in range(G // 2):
                    main_pair(sg, jp, sc_ps)
                if sg + 1 < NSG:
                    load_and_stats(sg + 1)

        nc.sync.dma_start(d_out_pl.rearrange("b d -> () (b d)"), stage_pl[:])

    nc.compile()
    return nc


def prep_inputs(nodes, pooled_edges, mask, globs, ln_g, ln_b,
                fw1, fb1, fw2, fb2, aw1, ab1, aw2, ab2, bpc=BPC, ncores=NCORES):
    """Host-side prep: fold LN affine into W1s, reshape biases/masks, cast."""
    f32 = np.float32
    bf = ml_dtypes.bfloat16
    ln_g = np.asarray(ln_g, f32)
    ln_b = np.asarray(ln_b, f32)
    fw1 = np.asarray(fw1, f32)
    aw1 = np.asarray(aw1, f32)

    fw1_eff = (fw1[:FEAT_IN] * ln_g[:, None]).astype(bf)
    fw1c = np.ascontiguousarray(fw1[FEAT_IN:]).astype(bf)
    fb1_eff = (np.asarray(fb1, f32) + ln_b @ fw1[:FEAT_IN]).astype(f32)
    aw1_eff = (aw1[:FEAT_IN] * ln_g[:, None]).astype(bf)
    aw1c = np.ascontiguousarray(aw1[FEAT_IN:]).astype(bf)
    ab1_eff = (np.asarray(ab1, f32) + ln_b @ aw1[:FEAT_IN]).astype(f32)
    # ab2 cancels exactly in the softmax; dropped.

    shared = {
        "w_fw1": fw1_eff,
        "w_fw1c": fw1c,
        "w_fw2": np.asarray(fw2, f32).astype(bf),
        "w_aw1": aw1_eff,
        "w_aw1c": aw1c,
        "w_aw2": np.asarray(aw2, f32).astype(bf),
        "b_f1": np.ascontiguousarray(fb1_eff.reshape(MF, 128).T),
        "b_f2": np.ascontiguousarray(np.asarray(fb2, f32).reshape(MO, 128).T),
        "b_a1": np.ascontiguousarray(ab1_eff.reshape(MA, 128).T),
        "ident": np.eye(128, dtype=bf),
        "ones_c": np.ones((128, 1), f32),
    }

    mask_t = np.ascontiguousarray(
        np.asarray(mask).astype(f32).reshape(-1, NT, 128).transpose(0, 2, 1)
    )
    globs = np.asarray(globs, f32)
    nodes = np.asarray(nodes, f32)
    pooled_edges = np.asarray(pooled_edges, f32)

    in_maps = []
    for c in range(ncores):
        s = slice(c * bpc, (c + 1) * bpc)
        m = dict(shared)
        m["nodes"] = np.ascontiguousarray(nodes[s])
        m["pedges"] = np.ascontiguousarray(pooled_edges[s])
        m["maskt"] = np.ascontiguousarray(mask_t[s])
        m["globsT"] = np.ascontiguousarray(globs[s].T).astype(bf)
        in_maps.append(m)
    return in_maps


_NC_CACHE = {}


def _get_nc(bpc=BPC, G=8):
    key = (bpc, G)
    if key not in _NC_CACHE:
        _NC_CACHE[key] = build_nc(bpc, G)
    return _NC_CACHE[key]


def kernel(nodes, pooled_edges, mask, globs, ln_g, ln_b,
           fw1, fb1, fw2, fb2, aw1, ab1, aw2, ab2):
    nc = _get_nc()
    in_maps = prep_inputs(nodes, pooled_edges, mask, globs, ln_g, ln_b,
                          fw1, fb1, fw2, fb2, aw1, ab1, aw2, ab2)
    res = run_bass_kernel_spmd(nc, in_maps, list(range(NCORES)))
    new_nodes = np.concatenate([r["out_nn"] for r in res.results], axis=0)
    pooled = np.concatenate([r["out_pl"] for r in res.results], axis=0)
    return new_nodes, pooled
